# revision 10
# baseline (speedup 1.0000x reference)
"""Circle Loss (PML-style) on 8 Trainium2 NeuronCores via Bass/Tile.

Full inputs -> full scalar output. Row-sharded: each core handles 1024 rows
of the 8192x8192 cosine-similarity matrix.

Fast path (inputs matching reference.setup_inputs structure: pos one-hot at
(i+N) mod 2N, neg = ~(pos|eye)): no masks are transferred at all. Per
element, new = fn = g*relu(D+1/4)*(D-1/4) = g*(v-1/2)*v with v=relu(D+1/4),
computed as ACT-Relu (or DVE max/add) from PSUM + one fused DVE
scalar_tensor_tensor. The two excluded entries per row (diagonal, positive
pair) sit on static block diagonals after a per-core column rotation and
are excised with a tiny tensor_tensor min against a host constant. Row max
via a DVE halving tree, then one [128, 8192] exp with per-row bias and
fused row-sum. Host does normalization, the positive-pair logit, softplus
and the nonzero mean.

Fallback (any other masks): the original mask-based kernel below.
"""

import sys

sys.path.insert(0, "/opt/trn_rl_repo")

import numpy as np

TWO_N = 8192
D_EMB = 256
N_CORES = 8
ROWS_PER_CORE = TWO_N // N_CORES  # 1024
RT = ROWS_PER_CORE // 128  # 8 row tiles per core
CHUNK = 2048
NCH = TWO_N // CHUNK  # 4 column chunks
B_SHIFT = 0.75
# fraction of chunks whose neg-image goes through ACT (Square) instead of
# the DVE route; balances the two engines (fallback kernel)
ACT_ROUTE = (True, False, True, False)

# fast path: per-chunk route for the PSUM->SBUF pass producing w = z^2,
# z = max(D, -1/4).  'A' = ACT Relu (v) + DVE shift (z) + DVE square;
# 'V' = DVE tensor_scalar max (z) + DVE square.  V first: DVE starts the
# row tile while ACT drains the previous tile's exp.
FAST_ROUTE = ("V", "A", "A", "A")

_RUN_KWARGS: dict = {}
_NC_CACHE: dict = {}


def _split_waits(nc, maxw=1):
    """walrus in this container accepts at most ~2 sem-waits per
    instruction (1 for ACTIVATE); split extras onto preceding NoOps."""
    import concourse.mybir as mybir

    n_new = 0
    for bb in nc.main_func.blocks:
        insts = bb.instructions
        i = 0
        while i < len(insts):
            ins = insts[i]
            si = ins.sync_info
            if si is not None and si.on_wait and len(si.on_wait) > maxw:
                waits = list(si.on_wait)
                ins.sync_info = mybir.SyncInfo(
                    on_wait=waits[:maxw], on_update=si.on_update
                )
                rest = waits[maxw:]
                pos = i
                while rest:
                    chunk, rest = rest[:maxw], rest[maxw:]
                    nop = mybir.InstNoOp(name=f"I-waitfix-{n_new}")
                    n_new += 1
                    nop.engine = ins.engine
                    nop.sync_info = mybir.SyncInfo(on_wait=chunk, on_update=[])
                    insts.insert(pos, nop)
                    pos += 1
                    i += 1
            i += 1
    return n_new


def _build_nc_fast():
    import concourse.bass as bass
    import concourse.tile as tile
    from concourse import mybir

    f32 = mybir.dt.float32
    f16 = mybir.dt.float16
    AF = mybir.ActivationFunctionType
    ALU = mybir.AluOpType

    nc = bass.Bass("TRN2", target_bir_lowering=False)

    def reg_const(val, dtype=f32):
        t = nc.alloc_sbuf_tensor(f"const-{dtype.name}-{val}", [128, 1], dtype)
        nc.gpsimd.memset(t.ap(), val)
        nc.const_aps.aps[(dtype, val)] = t.ap()

    reg_const(0.25)
    nc.all_engine_barrier()

    eT = nc.dram_tensor("eT", [D_EMB, TWO_N], f16, kind="ExternalInput")
    xmin_d = nc.dram_tensor("xmin", [128, 128], f16, kind="ExternalInput")
    m_out = nc.dram_tensor("m_all", [128, RT], f32, kind="ExternalOutput")
    s_out = nc.dram_tensor("s_all", [128, RT], f32, kind="ExternalOutput")

    with tile.TileContext(nc) as tc:
        with tc.tile_pool(name="singles", bufs=1) as singles, \
             tc.tile_pool(name="vpool", bufs=3) as vpool, \
             tc.tile_pool(name="wpool", bufs=2) as wpool, \
             tc.tile_pool(name="tpool", bufs=2) as tpool, \
             tc.tile_pool(name="small", bufs=4) as small, \
             tc.tile_pool(name="psum", bufs=2, space="PSUM") as psump:

            # eT in column slices, k0/k1 interleaved and first columns first
            # so the first matmul can start after ~2 small transfers
            e_sb = [
                singles.tile([128, TWO_N], f16, tag=f"e{k}", name=f"e{k}")
                for k in range(2)
            ]
            for c0, c1 in ((0, 512), (512, 2048), (2048, 5120), (5120, TWO_N)):
                for k in range(2):
                    nc.sync.dma_start(
                        out=e_sb[k][:, c0:c1],
                        in_=eT[k * 128:(k + 1) * 128, c0:c1],
                    )
            xmin = singles.tile([128, 128], f16, tag="xmin")
            nc.sync.dma_start(out=xmin, in_=xmin_d[:, :])

            m_all = singles.tile([128, RT], f32, tag="m_all")
            s_all = singles.tile([128, RT], f32, tag="s_all")

            for rt in range(RT):
                r0 = rt * 128
                wbig = wpool.tile([128, TWO_N], f16, tag="wbig")
                pm01 = tpool.tile([128, CHUNK], f16, tag="pm01")
                pm23 = tpool.tile([128, CHUNK], f16, tag="pm23")
                for ch in range(NCH):
                    c0 = ch * CHUNK
                    ps = psump.tile([128, CHUNK], f32, tag="ps")
                    for sub in range(CHUNK // 512):
                        s0 = sub * 512
                        for k in range(2):
                            nc.tensor.matmul(
                                ps[:, s0:s0 + 512],
                                e_sb[k][:, r0:r0 + 128],
                                e_sb[k][:, c0 + s0:c0 + s0 + 512],
                                start=(k == 0),
                                stop=(k == 1),
                            )
                    wsl = wbig[:, c0:c0 + CHUNK]
                    z = vpool.tile([128, CHUNK], f16, tag="z")
                    if FAST_ROUTE[ch] == "A":
                        v = vpool.tile([128, CHUNK], f16, tag="v")
                        nc.scalar.activation(v, ps, AF.Relu, bias=0.25, scale=1.0)
                        nc.vector.tensor_scalar_sub(z, v, 0.25)
                    else:
                        nc.vector.tensor_scalar(z, ps, -0.25, None, ALU.max)
                    nc.vector.tensor_tensor(out=wsl, in0=z, in1=z, op=ALU.mult)
                    if ch == 0 or ch == 2:
                        # excise diag (ch 0) / positive pair (ch 2): the
                        # excluded entries lie on the diagonal of the
                        # [128,128] block at column offset r0 within chunk
                        blk = wbig[:, c0 + r0:c0 + r0 + 128]
                        nc.vector.tensor_tensor(
                            out=blk, in0=blk, in1=xmin, op=ALU.min
                        )
                    # pairwise maxes start as soon as each pair is complete
                    if ch == 1:
                        nc.vector.tensor_tensor(
                            out=pm01, in0=wbig[:, 0:CHUNK],
                            in1=wbig[:, CHUNK:2 * CHUNK], op=ALU.max,
                        )
                    elif ch == 3:
                        nc.vector.tensor_tensor(
                            out=pm23, in0=wbig[:, 2 * CHUNK:3 * CHUNK],
                            in1=wbig[:, 3 * CHUNK:4 * CHUNK], op=ALU.max,
                        )

                # short max chain after the last square (high priority so the
                # exp's bias is ready as early as possible)
                with tc.high_priority(offset=-70):
                    nc.vector.tensor_tensor(
                        out=pm01[:, 0:1024], in0=pm01[:, 0:1024],
                        in1=pm01[:, 1024:2048], op=ALU.max,
                    )
                    nc.vector.tensor_tensor(
                        out=pm01[:, 0:1024], in0=pm01[:, 0:1024],
                        in1=pm23[:, 0:1024], op=ALU.max,
                    )
                    nc.vector.tensor_tensor(
                        out=pm01[:, 0:1024], in0=pm01[:, 0:1024],
                        in1=pm23[:, 1024:2048], op=ALU.max,
                    )
                    nc.vector.tensor_tensor(
                        out=pm01[:, 0:512], in0=pm01[:, 0:512],
                        in1=pm01[:, 512:1024], op=ALU.max,
                    )
                    nc.vector.tensor_tensor(
                        out=pm01[:, 0:256], in0=pm01[:, 0:256],
                        in1=pm01[:, 256:512], op=ALU.max,
                    )
                    msl = m_all[:, rt:rt + 1]
                    nc.vector.reduce_max(
                        msl, pm01[:, 0:256], axis=mybir.AxisListType.X
                    )
                    bias = small.tile([128, 1], f32, tag="bias")
                    nc.vector.tensor_scalar_mul(bias, msl, -256.0)
                    nc.scalar.activation(
                        wbig, wbig, AF.Exp, bias=bias[:, :], scale=256.0,
                        accum_out=s_all[:, rt:rt + 1],
                    )

            nc.sync.dma_start(out=m_out[:, :], in_=m_all)
            nc.sync.dma_start(out=s_out[:, :], in_=s_all)

    _split_waits(nc)
    return nc


def _is_structured(pm: np.ndarray, nm: np.ndarray) -> bool:
    if pm.shape != (TWO_N, TWO_N) or nm.shape != (TWO_N, TWO_N):
        return False
    idx = np.arange(TWO_N)
    pos_cols = (idx + TWO_N // 2) % TWO_N
    if not pm[idx, pos_cols].all():
        return False
    if int(pm.sum()) != TWO_N:
        return False
    expected_nm = ~pm.astype(bool)
    expected_nm[idx, idx] = False
    return bool(np.array_equal(nm.astype(bool), expected_nm))


def _kernel_fast(embeddings: np.ndarray) -> np.ndarray:
    from concourse.bass_utils import run_bass_kernel_spmd

    if "nc_fast" not in _NC_CACHE:
        _NC_CACHE["nc_fast"] = _build_nc_fast()
    nc = _NC_CACHE["nc_fast"]

    emb = np.asarray(embeddings, dtype=np.float32)
    e = emb / np.linalg.norm(emb.astype(np.float64), axis=1, keepdims=True)
    e = e.astype(np.float32)
    eT = np.ascontiguousarray(e.T).astype(np.float16)  # [256, 8192]

    xmin = np.full((128, 128), 60000.0, dtype=np.float16)
    np.fill_diagonal(xmin, -4.0)

    in_maps = []
    for c in range(N_CORES):
        eT_rot = np.ascontiguousarray(
            np.roll(eT, -c * ROWS_PER_CORE, axis=1)
        )
        in_maps.append({"eT": eT_rot, "xmin": xmin})

    res = run_bass_kernel_spmd(
        nc, in_maps, core_ids=list(range(N_CORES)), **_RUN_KWARGS
    )
    _NC_CACHE["last_result"] = res

    m_all = np.empty(TWO_N, dtype=np.float32)
    s_all = np.empty(TWO_N, dtype=np.float32)
    for c in range(N_CORES):
        mb = res.results[c]["m_all"]  # [128, RT]; row = c*1024 + rt*128 + p
        sb = res.results[c]["s_all"]
        m_all[c * ROWS_PER_CORE:(c + 1) * ROWS_PER_CORE] = mb.T.reshape(-1)
        s_all[c * ROWS_PER_CORE:(c + 1) * ROWS_PER_CORE] = sb.T.reshape(-1)

    # device works in w = z^2 = fn/gamma + 1/16; fold the 1/16 back here
    lse_n = 256.0 * (m_all.astype(np.float64) - 0.0625) + np.log(
        s_all.astype(np.float64)
    )

    dpos = np.einsum(
        "ij,ij->i", e.astype(np.float64), np.roll(e, -TWO_N // 2, axis=0).astype(np.float64)
    )
    lse_p = 256.0 * ((dpos - 1.0) ** 2 - 0.0625)

    lse = lse_p + lse_n
    losses = np.logaddexp(0.0, lse).astype(np.float32)
    nz = losses > 0
    cnt = int(nz.sum())
    if cnt == 0:
        return np.zeros((), dtype=np.float32)
    mean = np.float32(losses.sum(dtype=np.float32) / np.float32(max(cnt, 1)))
    return np.asarray(mean, dtype=np.float32)


def _build_nc(disjoint=False):
    import os
    import concourse.bass as bass
    import concourse.tile as tile
    from concourse import mybir

    no_inplace = os.environ.get("K_NOINPLACE", "0") == "1"
    no_gp = os.environ.get("K_NOGP", "0") == "1"
    no_exp = os.environ.get("K_NOEXP", "0") == "1"
    repeat = int(os.environ.get("K_REPEAT", "1"))

    f32 = mybir.dt.float32
    f16 = mybir.dt.float16
    AF = mybir.ActivationFunctionType
    ALU = mybir.AluOpType

    nc = bass.Bass("TRN2", target_bir_lowering=False)

    def reg_const(val, dtype=f32):
        t = nc.alloc_sbuf_tensor(f"const-{dtype.name}-{val}", [128, 1], dtype)
        nc.gpsimd.memset(t.ap(), val)
        nc.const_aps.aps[(dtype, val)] = t.ap()

    for vv in (-1.0, 0.25, -0.25):
        reg_const(vv)
    nc.all_engine_barrier()

    eT = nc.dram_tensor("eT", [D_EMB, TWO_N], f16, kind="ExternalInput")
    erT = nc.dram_tensor("erT", [D_EMB, ROWS_PER_CORE], f16, kind="ExternalInput")
    posm = nc.dram_tensor("posm", [ROWS_PER_CORE, TWO_N], f16, kind="ExternalInput")
    negm = nc.dram_tensor("negm", [ROWS_PER_CORE, TWO_N], f16, kind="ExternalInput")
    loss_out = nc.dram_tensor("loss", [128, RT], f32, kind="ExternalOutput")

    with tile.TileContext(nc) as tc:
        with tc.tile_pool(name="singles", bufs=1) as singles, \
             tc.tile_pool(name="chunks", bufs=3) as chunks, \
             tc.tile_pool(name="masks", bufs=2) as maskp, \
             tc.tile_pool(name="arow", bufs=6) as arowp, \
             tc.tile_pool(name="rmax", bufs=2) as rmaxp, \
             tc.tile_pool(name="small", bufs=4) as small, \
             tc.tile_pool(name="psum", bufs=2, space="PSUM") as psump:

            e_sb = []
            er_sb = []
            for k in range(2):
                t = singles.tile([128, TWO_N], f16, tag=f"e{k}")
                nc.sync.dma_start(out=t, in_=eT[k * 128:(k + 1) * 128, :])
                e_sb.append(t)
                tr = singles.tile([128, ROWS_PER_CORE], f16, tag=f"er{k}")
                nc.sync.dma_start(out=tr, in_=erT[k * 128:(k + 1) * 128, :])
                er_sb.append(tr)

            sp_all = singles.tile([128, RT], f32, tag="sp_all")
            sn_all = singles.tile([128, RT], f32, tag="sn_all")
            mp_all = singles.tile([128, RT], f32, tag="mp_all")
            mn_all = singles.tile([128, RT], f32, tag="mn_all")

            for rep in range(repeat):
              for rt in range(RT):
                r0 = rt * 128
                ap_cs = []
                an_cs = []
                rmp = rmaxp.tile([128, CHUNK], f16, tag="rmp")
                rmn = rmaxp.tile([128, CHUNK], f16, tag="rmn")
                for ch in range(NCH):
                    c0 = ch * CHUNK
                    ps = psump.tile([128, CHUNK], f32, tag="ps")
                    for sub in range(CHUNK // 512):
                        s0 = sub * 512
                        for k in range(2):
                            nc.tensor.matmul(
                                ps[:, s0:s0 + 512],
                                er_sb[k][:, r0:r0 + 128],
                                e_sb[k][:, c0 + s0:c0 + s0 + 512],
                                start=(k == 0),
                                stop=(k == 1),
                            )
                    qp = chunks.tile([128, CHUNK], f16, tag="qp")
                    nc.scalar.activation(qp, ps, AF.Square, bias=-1.0, scale=1.0)
                    v = chunks.tile([128, CHUNK], f16, tag="v")
                    nc.scalar.activation(v, ps, AF.Relu, bias=0.25, scale=1.0)

                    pos_t = maskp.tile([128, CHUNK], f16, tag="pos")
                    nc.sync.dma_start(out=pos_t, in_=posm[r0:r0 + 128, c0:c0 + CHUNK])
                    neg_t = maskp.tile([128, CHUNK], f16, tag="neg")
                    nc.sync.dma_start(out=neg_t, in_=negm[r0:r0 + 128, c0:c0 + CHUNK])

                    ap_c = arowp.tile([128, CHUNK], f16, tag="ap")
                    an_c = arowp.tile([128, CHUNK], f16, tag="an")
                    ap_cs.append(ap_c)
                    an_cs.append(an_c)
                    if disjoint:
                        # masks disjoint: nw|pos = a+B, nw|neg = b+B
                        aB = chunks.tile([128, CHUNK], f16, tag="aB")
                        nc.vector.tensor_scalar_add(aB, qp, B_SHIFT - 0.0625)
                        nc.vector.tensor_tensor(out=ap_c, in0=aB, in1=pos_t, op=ALU.mult)
                        bB = aB  # dead after ap_c
                        if ACT_ROUTE[ch]:
                            qvn = chunks.tile([128, CHUNK], f16, tag="qvn")
                            nc.scalar.activation(qvn, v, AF.Square, bias=-0.25, scale=1.0)
                            nc.vector.tensor_scalar_add(bB, qvn, B_SHIFT - 0.0625)
                        else:
                            # fn_hat = (v-1/2)*v; bB = fn_hat + B
                            t5 = chunks.tile([128, CHUNK], f16, tag="t5")
                            nc.vector.tensor_scalar_add(t5, v, -0.5)
                            u5 = chunks.tile([128, CHUNK], f16, tag="u5")
                            nc.vector.tensor_tensor(out=u5, in0=t5, in1=v, op=ALU.mult)
                            nc.vector.tensor_scalar_add(bB, u5, B_SHIFT)
                        if no_gp:
                            nc.vector.tensor_tensor(out=an_c, in0=bB, in1=neg_t, op=ALU.mult)
                        else:
                            nc.gpsimd.tensor_tensor(out=an_c, in0=bB, in1=neg_t, op=ALU.mult)
                    else:
                        # general: a = qp - 1/16, t1m = a*pos
                        a_t = chunks.tile([128, CHUNK], f16, tag="a_t")
                        nc.vector.tensor_scalar_add(a_t, qp, -0.0625)
                        pp = chunks.tile([128, CHUNK], f16, tag="pp")
                        nc.vector.tensor_tensor(out=pp, in0=a_t, in1=pos_t, op=ALU.mult)
                        fnm = qp  # reuse dead qp slot
                        qvn = chunks.tile([128, CHUNK], f16, tag="qvn")
                        nc.scalar.activation(qvn, v, AF.Square, bias=-0.25, scale=1.0)
                        b_t = a_t  # dead after pp
                        nc.vector.tensor_scalar_add(b_t, qvn, -0.0625)
                        nc.vector.tensor_tensor(out=fnm, in0=b_t, in1=neg_t, op=ALU.mult)
                        # nw = (pp + fnm) + B
                        s_t = a_t
                        nc.vector.tensor_tensor(out=s_t, in0=pp, in1=fnm, op=ALU.add)
                        nw = v  # reuse dead v slot
                        nc.vector.tensor_scalar_add(nw, s_t, B_SHIFT)
                        nc.vector.tensor_tensor(out=ap_c, in0=nw, in1=pos_t, op=ALU.mult)
                        if no_gp:
                            nc.vector.tensor_tensor(out=an_c, in0=nw, in1=neg_t, op=ALU.mult)
                        else:
                            nc.gpsimd.tensor_tensor(out=an_c, in0=nw, in1=neg_t, op=ALU.mult)
                    # running chunk-wise max
                    if ch == 0:
                        nc.vector.tensor_copy(out=rmp, in_=ap_c)
                        nc.vector.tensor_copy(out=rmn, in_=an_c)
                    else:
                        nc.vector.tensor_tensor(out=rmp, in0=rmp, in1=ap_c, op=ALU.max)
                        nc.vector.tensor_tensor(out=rmn, in0=rmn, in1=an_c, op=ALU.max)

                tail_prio = tc.high_priority(offset=-70)
                tail_prio.__enter__()
                mp = mp_all[:, rt:rt + 1]
                nc.vector.reduce_max(mp, rmp[:, :], axis=mybir.AxisListType.X)
                mn = mn_all[:, rt:rt + 1]
                nc.vector.reduce_max(mn, rmn[:, :], axis=mybir.AxisListType.X)
                bias_p = small.tile([128, 1], f32, tag="bias_p")
                nc.vector.tensor_scalar_mul(bias_p, mp, -256.0)
                bias_n = small.tile([128, 1], f32, tag="bias_n")
                nc.vector.tensor_scalar_mul(bias_n, mn, -256.0)
                # per-chunk in-place exp with fused row-sum parts
                sp_parts = small.tile([128, NCH], f32, tag="sp_parts")
                sn_parts = small.tile([128, NCH], f32, tag="sn_parts")
                for ch in range(NCH if not no_exp else 0):
                    nc.scalar.activation(
                        ap_cs[ch], ap_cs[ch], AF.Exp, bias=bias_p[:, :], scale=256.0,
                        accum_out=sp_parts[:, ch:ch + 1],
                    )
                    nc.scalar.activation(
                        an_cs[ch], an_cs[ch], AF.Exp, bias=bias_n[:, :], scale=256.0,
                        accum_out=sn_parts[:, ch:ch + 1],
                    )
                if not no_exp:
                    nc.vector.reduce_sum(
                        sp_all[:, rt:rt + 1], sp_parts[:, :], axis=mybir.AxisListType.X
                    )
                    nc.vector.reduce_sum(
                        sn_all[:, rt:rt + 1], sn_parts[:, :], axis=mybir.AxisListType.X
                    )
                    tail_prio.__exit__(None, None, None)
                else:
                    nc.vector.tensor_copy(out=sp_all[:, rt:rt + 1], in_=bias_p)
                    nc.vector.tensor_copy(out=sn_all[:, rt:rt + 1], in_=bias_n)
                    tail_prio.__exit__(None, None, None)

            # epilogue on [128, RT]
            lp = small.tile([128, RT], f32, tag="lp")
            nc.scalar.activation(lp, sp_all, AF.Ln, bias=0.0, scale=1.0)
            ln_ = small.tile([128, RT], f32, tag="ln")
            nc.scalar.activation(ln_, sn_all, AF.Ln, bias=0.0, scale=1.0)
            msum = small.tile([128, RT], f32, tag="msum")
            nc.vector.tensor_tensor(out=msum, in0=mp_all, in1=mn_all, op=ALU.add)
            m256 = small.tile([128, RT], f32, tag="m256")
            nc.vector.tensor_scalar(
                m256, msum, -2.0 * B_SHIFT, 256.0, ALU.add, ALU.mult
            )
            lsum = small.tile([128, RT], f32, tag="lsum")
            nc.vector.tensor_tensor(out=lsum, in0=lp, in1=ln_, op=ALU.add)
            lse = small.tile([128, RT], f32, tag="lse")
            nc.vector.tensor_tensor(out=lse, in0=m256, in1=lsum, op=ALU.add)
            # softplus(x) = max(x,0) + ln(1 + exp(-|x|))
            ax = small.tile([128, RT], f32, tag="ax")
            nc.scalar.activation(ax, lse, AF.Abs, bias=0.0, scale=1.0)
            et = small.tile([128, RT], f32, tag="et")
            nc.scalar.activation(et, ax, AF.Exp, bias=0.0, scale=-1.0)
            l1p = small.tile([128, RT], f32, tag="l1p")
            nc.scalar.activation(l1p, et, AF.Ln, bias=1.0, scale=1.0)
            rx = small.tile([128, RT], f32, tag="rx")
            nc.vector.tensor_scalar(rx, lse, 0.0, None, ALU.max)
            loss_t = small.tile([128, RT], f32, tag="loss")
            nc.vector.tensor_tensor(out=loss_t, in0=rx, in1=l1p, op=ALU.add)
            nc.sync.dma_start(out=loss_out[:, :], in_=loss_t)

    _split_waits(nc)
    return nc


def kernel(embeddings: np.ndarray, pos_mask: np.ndarray, neg_mask: np.ndarray) -> np.ndarray:
    import ml_dtypes  # noqa: F401
    from concourse.bass_utils import run_bass_kernel_spmd

    pm = np.asarray(pos_mask)
    nm = np.asarray(neg_mask)
    if _is_structured(pm, nm):
        return _kernel_fast(embeddings)

    disjoint = not bool(np.any(np.logical_and(pm, nm)))
    key = "nc_disjoint" if disjoint else "nc_general"
    if key not in _NC_CACHE:
        _NC_CACHE[key] = _build_nc(disjoint=disjoint)
    nc = _NC_CACHE[key]

    emb = np.asarray(embeddings, dtype=np.float32)
    e = emb / np.linalg.norm(emb.astype(np.float64), axis=1, keepdims=True)
    eT = np.ascontiguousarray(e.T).astype(np.float16)

    pos_f16 = pm.astype(np.float16)
    neg_f16 = nm.astype(np.float16)

    in_maps = []
    for c in range(N_CORES):
        r0 = c * ROWS_PER_CORE
        in_maps.append({
            "eT": eT,
            "erT": np.ascontiguousarray(eT[:, r0:r0 + ROWS_PER_CORE]),
            "posm": np.ascontiguousarray(pos_f16[r0:r0 + ROWS_PER_CORE]),
            "negm": np.ascontiguousarray(neg_f16[r0:r0 + ROWS_PER_CORE]),
        })

    res = run_bass_kernel_spmd(
        nc, in_maps, core_ids=list(range(N_CORES)), **_RUN_KWARGS
    )
    _NC_CACHE["last_result"] = res

    losses = np.empty(TWO_N, dtype=np.float32)
    for c in range(N_CORES):
        blk = res.results[c]["loss"]  # [128, RT], loss[p, rt] = row rt*128+p
        losses[c * ROWS_PER_CORE:(c + 1) * ROWS_PER_CORE] = blk.T.reshape(-1)

    valid = pm.any(axis=1) & nm.any(axis=1)
    losses = losses * valid.astype(np.float32)
    nz = losses > 0
    cnt = int(nz.sum())
    if cnt == 0:
        return np.zeros((), dtype=np.float32)
    mean = np.float32(losses.sum(dtype=np.float32) / np.float32(max(cnt, 1)))
    return np.asarray(mean, dtype=np.float32)
